# revision 6
# baseline (speedup 1.0000x reference)
"""Trainium2 Bass kernel for ContinuousAxialDW.

The reference op (continuous-offset axial depthwise conv, bilinear sampling)
collapses to two 1D depthwise convolutions with *integer* shifts, because the
bilinear fraction frac(off*r) is constant along the sampled axis:

    out[b,c,h,w] = x + sum_s A[c,s]*x[b,c,h+s,w] + sum_t B[c,t]*x[b,c,h,w+t]

with zero padding at the borders.  Folding the identity into the H-term this
is, per channel c:

    out[b,c] = Mh[c] @ X  +  X @ Sw[c]        (X = x[b,c], 256x256)

where Mh = I + banded(A), Sw = banded(B) are host-built 256x256 banded
matrices (band halfwidth bw, = 5 for r=1.5).  Both terms run on the
TensorEngine, exploiting bandedness to skip the zero blocks:

  * term1 = Mh @ X: one diagonal-block matmul per h-half (K=128), plus the
    two bw-wide corner couplings as a pair of 32x32 col/row-tiled matmuls
    (tile_position) that execute concurrently in disjoint PE quadrants.
  * term2 = X @ Sw: lhsT = (X^T) blocks via PE transpose; per w'-block the
    rhs streams only the N=128+bw live output columns (the straddle columns
    123..132 are accumulated by both blocks).

All device I/O and matmul operands are bf16 (tolerance is 2e-2; bf16 adds
~0.4% relative error): halves HBM traffic vs f32, enables FWL weight loads,
and speeds PE transposes.  PSUM accumulation stays f32; the X^T scratch stays
bf16 end-to-end so its PSUM->SBUF evacuation runs in the DVE 2x mode.

Sharding: channels across the 8 cores (12 ch/core, all 8 batch images), so the
per-channel banded matrices are DMA'd once and reused across 8 images.
"""

import os
import sys

import numpy as np

for _p in ("/opt/trn_rl_repo", "/root/.axon_site/_ro/trn_rl_repo"):
    if _p not in sys.path and os.path.isdir(_p):
        sys.path.append(_p)

import ml_dtypes

import concourse.bass as bass
import concourse.mybir as mybir
from concourse import bacc, tile
from concourse.bass_utils import run_bass_kernel_spmd

N_CORES = 8
B, C, H, W = 8, 96, 256, 256
C_LOC = C // N_CORES  # 12 channels per core
KTAPS = 7

F32 = mybir.dt.float32
BF16 = mybir.dt.bfloat16
NP_BF16 = ml_dtypes.bfloat16

# run_bass_kernel_spmd results of the most recent kernel() call (for test
# harness introspection: exec_time_ns when BASS_TRACE=1).
LAST_RESULTS = None

_PROGRAMS = {}  # band halfwidth -> cached Bass program


def _emit(tc, x_d, m_d, i_d, o_d, bw):
    """Emit the per-core program.

    Per-core DRAM tensors (partition-major: leading dim 128 = h%128 resp.
    matrix row):
      x_d: [128, C_LOC, 2(hb), B, W] bf16 input shard
      m_d: [128, C_LOC*4*256] bf16 per-channel banded matrices, 4 chunks:
           m=0,1: MhT rows 0:128 / 128:256   (lhsT for term1)
           m=2,3: Sw  rows 0:128 / 128:256   (rhs for term2)
      i_d: [128, 128] bf16 identity (for PE transposes)
      o_d: [128, C_LOC, 2(hb), B, W] bf16 output shard
    """
    nc = tc.nc
    n_pairs = 4 * C_LOC  # global pair index g = c*4 + p
    n2 = 128 + bw  # live output columns per w'-block in term2
    with (
        tc.tile_pool(name="const", bufs=1) as cpool,
        tc.tile_pool(name="xin", bufs=3) as xpool,
        tc.tile_pool(name="xtp", bufs=3) as xtpool,
        tc.tile_pool(name="outp", bufs=2) as opool,
        tc.tile_pool(name="psx", bufs=3, space="PSUM") as psx,
        tc.tile_pool(name="pso", bufs=2, space="PSUM") as pso,
    ):
        ident = cpool.tile([128, 128], BF16, name="ident")
        nc.sync.dma_start(ident[:], i_d[:])
        mats = cpool.tile([128, C_LOC * 4 * 256], BF16, name="mats")

        def mat_sl(c, m, lo, n, p0=0, p1=128):
            base = (c * 4 + m) * 256 + lo
            return mats[p0:p1, base : base + n]

        chans = {}  # c -> (xt_, ot_)
        pairs = {}  # g -> [pxt, xw]
        outs = {}  # g -> po

        def start_channel(c):
            # x loads on SP (HWDGE), banded-matrix loads on ACT's HWDGE queue
            # (so they don't serialize behind the x stream), stores on gpsimd
            # (SWDGE).  One contiguous 1 MiB DMA per channel for x/out.
            if c % 2 == 0:
                nc.scalar.dma_start(
                    mats[:, c * 1024 : (c + 2) * 1024],
                    m_d[:, c * 1024 : (c + 2) * 1024],
                )
            xt_ = xpool.tile([128, 2, 2048], BF16, name=f"x{c}", tag="x")
            if c == 0:
                # split the first channel's load per pair so the PE can start
                # on pair 0 after 256 KiB instead of 1 MiB
                for p in range(4):
                    nc.sync.dma_start(
                        xt_[:, :, p * 512 : p * 512 + 512],
                        x_d[:, c, :, 2 * p : 2 * p + 2, :],
                    )
            else:
                nc.sync.dma_start(xt_[:], x_d[:, c])
            ot_ = opool.tile([128, 2, 2048], BF16, name=f"o{c}", tag="o")
            chans[c] = (xt_, ot_)

        def tr_half(g, wb):
            # 4 PE transposes building the X^T w-block wb for pair g; after
            # the second block, one wide evacuation (DVE/ACT alternating).
            c, p = divmod(g, 4)
            xt_, _ = chans[c]
            if wb == 0:
                pxt = psx.tile([128, 1024], BF16, name=f"pxt{g}", tag="pxt")
                pairs[g] = [pxt, None]
            pxt = pairs[g][0]
            for bi in range(2):
                b = 2 * p + bi
                for hb in range(2):
                    nc.tensor.transpose(
                        pxt[:, wb * 512 + bi * 256 + hb * 128 : wb * 512 + bi * 256 + hb * 128 + 128],
                        xt_[:, hb, b * 256 + wb * 128 : b * 256 + wb * 128 + 128],
                        ident[:],
                    )
            if wb == 1:
                xw = xtpool.tile([128, 1024], BF16, name=f"xw{g}", tag="xt")
                pairs[g][1] = xw
                if g % 2 == 0:
                    nc.vector.tensor_copy(xw[:], pxt[:])
                else:
                    nc.scalar.copy(xw[:], pxt[:])

        def terms_group(g, hb):
            # accumulation group for the h-half hb of pair g: one term1
            # diagonal-block matmul (N=512) + four term2 banded matmuls
            # (N=128+bw).
            c, p = divmod(g, 4)
            xt_, _ = chans[c]
            xw = pairs[g][1]
            if hb == 0:
                outs[g] = pso.tile([128, 2, 512], F32, name=f"po{g}", tag="po")
            po = outs[g]
            nc.tensor.matmul(
                po[:, hb, :],
                lhsT=mat_sl(c, hb, hb * 128, 128),
                rhs=xt_[:, hb, p * 512 : p * 512 + 512],
                start=True,
                stop=False,
            )
            for wb in range(2):  # term2: X @ Sw via lhsT = X^T blocks
                lo = 0 if wb == 0 else 128 - bw
                for bi in range(2):
                    nc.tensor.matmul(
                        po[:, hb, bi * 256 + lo : bi * 256 + lo + n2],
                        lhsT=xw[:, wb * 512 + bi * 256 + hb * 128 : wb * 512 + bi * 256 + hb * 128 + 128],
                        rhs=mat_sl(c, 2 + wb, lo, n2),
                        start=False,
                        stop=False,
                    )

        def corners_and_evac(g):
            # term1 corner couplings across the h=128 boundary: two 32x32
            # matmuls in disjoint PE quadrants (concurrent), then one wide
            # PSUM->SBUF evacuation of the pair's full [128, 1024] output.
            c, p = divmod(g, 4)
            xt_, ot_ = chans[c]
            po = outs[g]
            # out rows 128-bw..127 (h-half 0) += MhT[128:160]^T rows coupling
            nc.tensor.matmul(
                po[96:128, 0, :],
                lhsT=mat_sl(c, 1, 96, 32, 0, 32),
                rhs=xt_[0:32, 1, p * 512 : p * 512 + 512],
                start=False,
                stop=True,
                tile_position=(0, 96),
            )
            # out rows 128..128+bw-1 (h-half 1) += coupling from rows <128
            nc.tensor.matmul(
                po[0:32, 1, :],
                lhsT=mat_sl(c, 0, 128, 32, 96, 128),
                rhs=xt_[96:128, 0, p * 512 : p * 512 + 512],
                start=False,
                stop=True,
                tile_position=(96, 0),
            )
            if g == n_pairs - 1:
                # final pair: split the evacuation across both engines to
                # shorten the kernel tail
                nc.vector.tensor_copy(ot_[:, 0, p * 512 : p * 512 + 512], po[:, 0, :])
                nc.scalar.copy(ot_[:, 1, p * 512 : p * 512 + 512], po[:, 1, :])
            elif g % 2 == 0:
                nc.scalar.copy(ot_[:, :, p * 512 : p * 512 + 512], po[:])
            else:
                nc.vector.tensor_copy(ot_[:, :, p * 512 : p * 512 + 512], po[:])
            del pairs[g], outs[g]
            if c == C_LOC - 1:
                # last channel: store per pair so the tail only waits on the
                # final 256 KiB
                nc.gpsimd.dma_start(
                    o_d[:, c, :, 2 * p : 2 * p + 2, :],
                    ot_[:, :, p * 512 : p * 512 + 512],
                )
            elif p == 3:  # channel done: store (off the SP engine)
                nc.gpsimd.dma_start(o_d[:, c], ot_[:])

        # software pipeline: pair g's transposes are interleaved between pair
        # g-1's two matmul groups, so TensorE always has real matmuls in every
        # HAM window and the X^T evac latency is hidden one pair ahead.
        for g in range(n_pairs + 1):
            if g < n_pairs:
                c, p = divmod(g, 4)
                if p == 0:
                    start_channel(c)
                tr_half(g, 0)
            if g > 0:
                terms_group(g - 1, 0)
            if g < n_pairs:
                tr_half(g, 1)
            if g > 0:
                terms_group(g - 1, 1)
                corners_and_evac(g - 1)


def _build_program(bw):
    if bw in _PROGRAMS:
        return _PROGRAMS[bw]
    nc = bacc.Bacc("TRN2", target_bir_lowering=False, debug=False, num_devices=N_CORES)
    x_d = nc.dram_tensor("x_sh", [128, C_LOC, 2, B, W], BF16, kind="ExternalInput").ap()
    m_d = nc.dram_tensor("mats", [128, C_LOC * 4 * 256], BF16, kind="ExternalInput").ap()
    i_d = nc.dram_tensor("ident", [128, 128], BF16, kind="ExternalInput").ap()
    o_d = nc.dram_tensor("out_sh", [128, C_LOC, 2, B, W], BF16, kind="ExternalOutput").ap()
    with tile.TileContext(nc) as tc:
        _emit(tc, x_d, m_d, i_d, o_d, bw)
    nc.compile()
    _PROGRAMS[bw] = nc
    return nc


def _eff_coeffs(taps, r):
    """taps: [k, C] per-tap depthwise weights -> dict integer_shift -> coeff[C].

    Mirrors the reference: pos = coord + off*r (f32), i0 = floor(pos),
    frac = pos - i0; both are constant per tap since coord is integral.
    """
    r_val = max(float(np.float32(r)), 1.0)
    k = taps.shape[0]
    pad = k // 2
    coeffs = {}
    for i, off in enumerate(range(-pad, pad + 1)):
        pos = np.float32(off * np.float32(r_val))
        s0 = int(np.floor(pos))
        f = float(np.float32(pos)) - s0
        for s, cmul in ((s0, 1.0 - f), (s0 + 1, f)):
            if cmul != 0.0:
                acc = coeffs.setdefault(s, np.zeros(taps.shape[1], np.float64))
                acc += cmul * taps[i].astype(np.float64)
    return coeffs


def _build_mats(weight_h, weight_w, r):
    """Host-build per-channel banded matrices, chunked for the kernel.

    Returns ([C, 4, 128, 256] f32, bw): per channel the two 128-row chunks of
    MhT = (I + banded_h)^T followed by the two chunks of Sw = banded_w,
    where (banded)[h, h+s] = A[c, s] i.e. MhT[h+s, h] = A[c, s], and
    Sw[w+t, w] = B[c, t]; bw is the band halfwidth max|s|.
    """
    ch = _eff_coeffs(weight_h[:, 0, :, 0].T, r)
    cw = _eff_coeffs(weight_w[:, 0, 0, :].T, r)
    bw = max(max(abs(s) for s in ch), max(abs(t) for t in cw))
    mh_t = np.zeros((C, H, H), np.float64)
    mh_t[:, np.arange(H), np.arange(H)] = 1.0
    for s, coef in ch.items():
        i = np.arange(max(0, s), H + min(0, s))
        mh_t[:, i, i - s] += coef[:, None]
    sw = np.zeros((C, W, W), np.float64)
    for t, coef in cw.items():
        i = np.arange(max(0, t), W + min(0, t))
        sw[:, i, i - t] += coef[:, None]
    mats = np.empty((C, 4, 128, 256), np.float32)
    mats[:, 0] = mh_t[:, 0:128, :]
    mats[:, 1] = mh_t[:, 128:256, :]
    mats[:, 2] = sw[:, 0:128, :]
    mats[:, 3] = sw[:, 128:256, :]
    return mats, bw


def kernel(**inputs):
    global LAST_RESULTS
    x = np.asarray(inputs["x"], dtype=np.float32)
    weight_h = np.asarray(inputs["weight_h"], dtype=np.float32)
    weight_w = np.asarray(inputs["weight_w"], dtype=np.float32)
    r = np.asarray(inputs["r"], dtype=np.float32)
    assert x.shape == (B, C, H, W), x.shape

    mats, bw = _build_mats(weight_h, weight_w, r)  # [C, 4, 128, 256]
    assert 0 < bw <= 32, bw  # corner couplings use one 32x32 quadrant
    # [C, 4, 128, 256] -> [128, C, 4, 256] (partition-major), bf16
    mats_p = np.ascontiguousarray(mats.transpose(2, 0, 1, 3)).astype(NP_BF16)
    mats_p = mats_p.reshape(128, C * 4 * 256)
    ident = np.eye(128, dtype=NP_BF16)

    # [B, C, H, W] -> [128(h%128), C, 2(hb), B, W] bf16 (partition-major)
    xs = np.ascontiguousarray(
        x.reshape(B, C, 2, 128, W).transpose(3, 1, 2, 0, 4)
    ).astype(NP_BF16)

    nc = _build_program(bw)
    in_maps = [
        {
            "x_sh": np.ascontiguousarray(xs[:, i * C_LOC : (i + 1) * C_LOC]),
            "mats": np.ascontiguousarray(
                mats_p[:, i * C_LOC * 1024 : (i + 1) * C_LOC * 1024]
            ),
            "ident": ident,
        }
        for i in range(N_CORES)
    ]
    res = run_bass_kernel_spmd(nc, in_maps, list(range(N_CORES)))
    LAST_RESULTS = res
    # [128, C_LOC, 2, B, W] bf16 per core -> [B, C, H, W] f32
    o = np.concatenate([res.results[i]["out_sh"] for i in range(N_CORES)], axis=1)
    out = o.transpose(3, 1, 2, 0, 4).reshape(B, C, H, W)
    return np.ascontiguousarray(out).astype(np.float32)


# revision 9
# speedup vs baseline: 1.1164x; 1.1164x over previous
"""Trainium2 Bass kernel for ContinuousAxialDW.

The reference op (continuous-offset axial depthwise conv, bilinear sampling)
collapses to two 1D depthwise convolutions with *integer* shifts, because the
bilinear fraction frac(off*r) is constant along the sampled axis:

    out[b,c,h,w] = x + sum_s A[c,s]*x[b,c,h+s,w] + sum_t B[c,t]*x[b,c,h,w+t]

with zero padding at the borders.  Folding the identity into the H-term this
is, per channel c:

    out[b,c] = Mh[c] @ X  +  X @ Sw[c]        (X = x[b,c], 256x256)

where Mh = I + banded(A), Sw = banded(B) are host-built 256x256 banded
matrices (band halfwidth bw, = 5 for r=1.5).  Both terms run on the
TensorEngine, exploiting bandedness to skip the zero blocks:

  * term1 = Mh @ X: one diagonal-block matmul per h-half (K=128), plus the
    two bw-wide corner couplings as a pair of 32x32 col/row-tiled matmuls
    (tile_position) that execute concurrently in disjoint PE quadrants.
  * term2 = X @ Sw: lhsT = (X^T) blocks via PE transpose; per w'-block the
    rhs streams only the N=128+bw live output columns (the straddle columns
    123..132 are accumulated by both blocks).

All device I/O and matmul operands are bf16 (tolerance is 2e-2; bf16 adds
~0.4% relative error): halves HBM traffic vs f32, enables FWL weight loads,
and speeds PE transposes.  PSUM accumulation stays f32; the X^T scratch stays
bf16 end-to-end so its PSUM->SBUF evacuation runs in the DVE 2x mode.

Sharding: channels across the 8 cores (12 ch/core, all 8 batch images), so the
per-channel banded matrices are DMA'd once and reused across 8 images.
"""

import os
import sys

import numpy as np

for _p in ("/opt/trn_rl_repo", "/root/.axon_site/_ro/trn_rl_repo"):
    if _p not in sys.path and os.path.isdir(_p):
        sys.path.append(_p)

import ml_dtypes

import concourse.bass as bass
import concourse.mybir as mybir
from concourse import bacc, tile
from concourse.bass_utils import run_bass_kernel_spmd

N_CORES = 8
B, C, H, W = 8, 96, 256, 256
C_LOC = C // N_CORES  # 12 channels per core
KTAPS = 7

F32 = mybir.dt.float32
BF16 = mybir.dt.bfloat16
NP_BF16 = ml_dtypes.bfloat16

# run_bass_kernel_spmd results of the most recent kernel() call (for test
# harness introspection: exec_time_ns when BASS_TRACE=1).
LAST_RESULTS = None

_PROGRAMS = {}  # band halfwidth -> cached Bass program


def _emit(tc, x_d, m_d, i_d, o_d, bw):
    """Emit the per-core program.

    Per-core DRAM tensors (partition-major: leading dim 128 = h%128 resp.
    matrix row):
      x_d: [128, C_LOC, 2(hb), B, W] bf16 input shard
      m_d: [128, C_LOC*4*256] bf16 per-channel banded matrices, 4 chunks:
           m=0,1: MhT rows 0:128 / 128:256   (lhsT for term1)
           m=2,3: Sw  rows 0:128 / 128:256   (rhs for term2)
      i_d: [128, 128] bf16 identity (for PE transposes)
      o_d: [128, C_LOC, 2(hb), B, W] bf16 output shard
    """
    nc = tc.nc
    n_pairs = 4 * C_LOC  # global pair index g = c*4 + p
    n2 = 128 + bw  # live output columns per w'-block in term2
    with (
        tc.tile_pool(name="const", bufs=1) as cpool,
        tc.tile_pool(name="xin", bufs=4) as xpool,
        tc.tile_pool(name="xtp", bufs=3) as xtpool,
        tc.tile_pool(name="outp", bufs=2) as opool,
        tc.tile_pool(name="psx", bufs=2, space="PSUM") as psx,
        tc.tile_pool(name="pso", bufs=3, space="PSUM") as pso,
    ):
        ident = cpool.tile([128, 128], BF16, name="ident")
        nc.sync.dma_start(ident[:], i_d[:])
        mats = cpool.tile([128, C_LOC * 4 * 256], BF16, name="mats")

        def mat_sl(c, m, lo, n, p0=0, p1=128):
            base = (c * 4 + m) * 256 + lo
            return mats[p0:p1, base : base + n]

        chans = {}  # c -> (xt_, ot_)
        pairs = {}  # g -> [pxt, xw]
        outs = {}  # g -> po

        def start_channel(c):
            # x loads on SP (HWDGE), banded-matrix loads on ACT's HWDGE queue
            # (so they don't serialize behind the x stream), stores on gpsimd
            # (SWDGE).  One contiguous 1 MiB DMA per channel for x/out.
            if c % 2 == 0:
                nc.scalar.dma_start(
                    mats[:, c * 1024 : (c + 2) * 1024],
                    m_d[:, c * 1024 : (c + 2) * 1024],
                )
            xt_ = xpool.tile([128, 2, 2048], BF16, name=f"x{c}", tag="x")
            nc.sync.dma_start(xt_[:], x_d[:, c])
            ot_ = opool.tile([128, 2, 2048], BF16, name=f"o{c}", tag="o")
            chans[c] = (xt_, ot_)

        def tr_half(g, wb):
            # 4 PE transposes building the X^T w-block wb for pair g; after
            # the second block, one wide evacuation (DVE/ACT alternating).
            c, p = divmod(g, 4)
            xt_, _ = chans[c]
            if wb == 0:
                pxt = psx.tile([128, 1024], BF16, name=f"pxt{g}", tag="pxt")
                pairs[g] = [pxt, None]
            pxt = pairs[g][0]
            for bi in range(2):
                b = 2 * p + bi
                for hb in range(2):
                    nc.tensor.transpose(
                        pxt[:, wb * 512 + bi * 256 + hb * 128 : wb * 512 + bi * 256 + hb * 128 + 128],
                        xt_[:, hb, b * 256 + wb * 128 : b * 256 + wb * 128 + 128],
                        ident[:],
                    )
            if wb == 1:
                xw = xtpool.tile([128, 1024], BF16, name=f"xw{g}", tag="xt")
                pairs[g][1] = xw
                if g % 2 == 0:
                    nc.vector.tensor_copy(xw[:], pxt[:])
                else:
                    nc.scalar.copy(xw[:], pxt[:])

        def terms_group(g, hb):
            # accumulation group for the h-half hb of pair g: one term1
            # diagonal-block matmul (N=512) + four term2 banded matmuls
            # (N=128+bw).
            c, p = divmod(g, 4)
            xt_, _ = chans[c]
            xw = pairs[g][1]
            if hb == 0:
                outs[g] = pso.tile([128, 2, 512], F32, name=f"po{g}", tag="po")
            po = outs[g]
            nc.tensor.matmul(
                po[:, hb, :],
                lhsT=mat_sl(c, hb, hb * 128, 128),
                rhs=xt_[:, hb, p * 512 : p * 512 + 512],
                start=True,
                stop=False,
            )
            for wb in range(2):  # term2: X @ Sw via lhsT = X^T blocks
                lo = 0 if wb == 0 else 128 - bw
                for bi in range(2):
                    nc.tensor.matmul(
                        po[:, hb, bi * 256 + lo : bi * 256 + lo + n2],
                        lhsT=xw[:, wb * 512 + bi * 256 + hb * 128 : wb * 512 + bi * 256 + hb * 128 + 128],
                        rhs=mat_sl(c, 2 + wb, lo, n2),
                        start=False,
                        stop=False,
                    )

        def corners_and_evac(g):
            # term1 corner couplings across the h=128 boundary: two 32x32
            # matmuls in disjoint PE quadrants (concurrent), then one wide
            # PSUM->SBUF evacuation of the pair's full [128, 1024] output.
            c, p = divmod(g, 4)
            xt_, ot_ = chans[c]
            po = outs[g]
            # out rows 128-bw..127 (h-half 0) += MhT[128:160]^T rows coupling
            nc.tensor.matmul(
                po[96:128, 0, :],
                lhsT=mat_sl(c, 1, 96, 32, 0, 32),
                rhs=xt_[0:32, 1, p * 512 : p * 512 + 512],
                start=False,
                stop=True,
                tile_position=(0, 96),
            )
            # out rows 128..128+bw-1 (h-half 1) += coupling from rows <128
            nc.tensor.matmul(
                po[0:32, 1, :],
                lhsT=mat_sl(c, 0, 128, 32, 96, 128),
                rhs=xt_[96:128, 0, p * 512 : p * 512 + 512],
                start=False,
                stop=True,
                tile_position=(96, 0),
            )
            if g == n_pairs - 1:
                # final pair: split the evacuation across both engines to
                # shorten the kernel tail
                nc.vector.tensor_copy(ot_[:, 0, p * 512 : p * 512 + 512], po[:, 0, :])
                nc.scalar.copy(ot_[:, 1, p * 512 : p * 512 + 512], po[:, 1, :])
            elif g % 2 == 0:
                nc.scalar.copy(ot_[:, :, p * 512 : p * 512 + 512], po[:])
            else:
                nc.vector.tensor_copy(ot_[:, :, p * 512 : p * 512 + 512], po[:])
            del pairs[g], outs[g]
            if c == C_LOC - 1 and p % 2 == 1:
                # last channel: store in halves so the tail only waits on the
                # final 512 KiB
                nc.gpsimd.dma_start(
                    o_d[:, c, :, 2 * p - 2 : 2 * p + 2, :],
                    ot_[:, :, p * 512 - 512 : p * 512 + 512],
                )
            elif c < C_LOC - 1 and p == 3:  # channel done: store
                nc.gpsimd.dma_start(o_d[:, c], ot_[:])

        # software pipeline: pair g's transposes are interleaved between pair
        # g-1's two matmul groups, so TensorE always has real matmuls in every
        # HAM window and the X^T evac latency is hidden one pair ahead.
        for g in range(n_pairs + 1):
            if g < n_pairs:
                c, p = divmod(g, 4)
                if p == 0:
                    start_channel(c)
                tr_half(g, 0)
            if g > 0:
                terms_group(g - 1, 0)
            if g < n_pairs:
                tr_half(g, 1)
            if g > 0:
                terms_group(g - 1, 1)
                corners_and_evac(g - 1)


def _build_program(bw):
    if bw in _PROGRAMS:
        return _PROGRAMS[bw]
    nc = bacc.Bacc("TRN2", target_bir_lowering=False, debug=False, num_devices=N_CORES)
    x_d = nc.dram_tensor("x_sh", [128, C_LOC, 2, B, W], BF16, kind="ExternalInput").ap()
    m_d = nc.dram_tensor("mats", [128, C_LOC * 4 * 256], BF16, kind="ExternalInput").ap()
    i_d = nc.dram_tensor("ident", [128, 128], BF16, kind="ExternalInput").ap()
    o_d = nc.dram_tensor("out_sh", [128, C_LOC, 2, B, W], BF16, kind="ExternalOutput").ap()
    with tile.TileContext(nc) as tc:
        _emit(tc, x_d, m_d, i_d, o_d, bw)
    nc.compile()
    _PROGRAMS[bw] = nc
    return nc


def _eff_coeffs(taps, r):
    """taps: [k, C] per-tap depthwise weights -> dict integer_shift -> coeff[C].

    Mirrors the reference: pos = coord + off*r (f32), i0 = floor(pos),
    frac = pos - i0; both are constant per tap since coord is integral.
    """
    r_val = max(float(np.float32(r)), 1.0)
    k = taps.shape[0]
    pad = k // 2
    coeffs = {}
    for i, off in enumerate(range(-pad, pad + 1)):
        pos = np.float32(off * np.float32(r_val))
        s0 = int(np.floor(pos))
        f = float(np.float32(pos)) - s0
        for s, cmul in ((s0, 1.0 - f), (s0 + 1, f)):
            if cmul != 0.0:
                acc = coeffs.setdefault(s, np.zeros(taps.shape[1], np.float64))
                acc += cmul * taps[i].astype(np.float64)
    return coeffs


def _build_mats(weight_h, weight_w, r):
    """Host-build per-channel banded matrices, chunked for the kernel.

    Returns ([C, 4, 128, 256] f32, bw): per channel the two 128-row chunks of
    MhT = (I + banded_h)^T followed by the two chunks of Sw = banded_w,
    where (banded)[h, h+s] = A[c, s] i.e. MhT[h+s, h] = A[c, s], and
    Sw[w+t, w] = B[c, t]; bw is the band halfwidth max|s|.
    """
    ch = _eff_coeffs(weight_h[:, 0, :, 0].T, r)
    cw = _eff_coeffs(weight_w[:, 0, 0, :].T, r)
    bw = max(max(abs(s) for s in ch), max(abs(t) for t in cw))
    mh_t = np.zeros((C, H, H), np.float64)
    mh_t[:, np.arange(H), np.arange(H)] = 1.0
    for s, coef in ch.items():
        i = np.arange(max(0, s), H + min(0, s))
        mh_t[:, i, i - s] += coef[:, None]
    sw = np.zeros((C, W, W), np.float64)
    for t, coef in cw.items():
        i = np.arange(max(0, t), W + min(0, t))
        sw[:, i, i - t] += coef[:, None]
    mats = np.empty((C, 4, 128, 256), np.float32)
    mats[:, 0] = mh_t[:, 0:128, :]
    mats[:, 1] = mh_t[:, 128:256, :]
    mats[:, 2] = sw[:, 0:128, :]
    mats[:, 3] = sw[:, 128:256, :]
    return mats, bw


def kernel(**inputs):
    global LAST_RESULTS
    x = np.asarray(inputs["x"], dtype=np.float32)
    weight_h = np.asarray(inputs["weight_h"], dtype=np.float32)
    weight_w = np.asarray(inputs["weight_w"], dtype=np.float32)
    r = np.asarray(inputs["r"], dtype=np.float32)
    assert x.shape == (B, C, H, W), x.shape

    mats, bw = _build_mats(weight_h, weight_w, r)  # [C, 4, 128, 256]
    assert 0 < bw <= 32, bw  # corner couplings use one 32x32 quadrant
    # [C, 4, 128, 256] -> [128, C, 4, 256] (partition-major), bf16
    mats_p = np.ascontiguousarray(mats.transpose(2, 0, 1, 3)).astype(NP_BF16)
    mats_p = mats_p.reshape(128, C * 4 * 256)
    ident = np.eye(128, dtype=NP_BF16)

    # [B, C, H, W] -> [128(h%128), C, 2(hb), B, W] bf16 (partition-major)
    xs = np.ascontiguousarray(
        x.reshape(B, C, 2, 128, W).transpose(3, 1, 2, 0, 4)
    ).astype(NP_BF16)

    nc = _build_program(bw)
    in_maps = [
        {
            "x_sh": np.ascontiguousarray(xs[:, i * C_LOC : (i + 1) * C_LOC]),
            "mats": np.ascontiguousarray(
                mats_p[:, i * C_LOC * 1024 : (i + 1) * C_LOC * 1024]
            ),
            "ident": ident,
        }
        for i in range(N_CORES)
    ]
    res = run_bass_kernel_spmd(nc, in_maps, list(range(N_CORES)))
    LAST_RESULTS = res
    # [128, C_LOC, 2, B, W] bf16 per core -> [B, C, H, W] f32
    o = np.concatenate([res.results[i]["out_sh"] for i in range(N_CORES)], axis=1)
    out = o.transpose(3, 1, 2, 0, 4).reshape(B, C, H, W)
    return np.ascontiguousarray(out).astype(np.float32)
